# revision 3
# baseline (speedup 1.0000x reference)
"""Trainium2 Bass kernel for nn_MediumRangeEdge (retrieval_knn).

Computes, for each batch graph: L2-normalize node features, pairwise
distance matrix dist = sq_n + sq_m - 2*x@x.T + relative_pos + INF*mask,
top-10 nearest (smallest dist) neighbor indices per node, then the edge
list [dst, src, 0].

Distribution: data-parallel over batch. 32 graphs -> 8 NeuronCores, 4
graphs per core. No cross-core communication.

Device-side math per graph (n = query node row, m = candidate column):
    score[n, m] = xh@xh.T[n, m] - cbias[n, m]
with host-precomputed cbias[b, n, m] = (rel[n,m] + INF*mask[n,m] + sq[b,m]) / 2.
score = (-dist + sq_n)/2; the row-constant sq_n/2 does not change the
per-row ordering, so top-10 of score == top-10 of -dist. Top-10 per row
is done on the DVE with max8 / max_index / match_replace (8+2).
"""

import sys

if "/opt/trn_rl_repo" not in sys.path:
    sys.path.insert(0, "/opt/trn_rl_repo")

import numpy as np

BATCH = 32
N = 784  # 28*28 nodes
D = 512
K = 10
RES = 28
INF = 100000.0
NCORES = 8
BPC = BATCH // NCORES  # graphs per core

P = 128
N_PT = 7  # partition tiles over N: 6*128 + 16
ROWS = [128, 128, 128, 128, 128, 128, 16]
HALF = 392  # free-dim split of N for PSUM (2 * 392 = 784)

_CACHE = {}


def _mask_np():
    idx = np.arange(N)
    r, c = idx // RES, idx % RES
    mask = np.zeros((N, N), np.float32)
    for dr, dc in [(0, -1), (0, 1), (-1, 0), (1, 0), (-1, -1), (-1, 1), (1, -1), (1, 1)]:
        rr, cc = r + dr, c + dc
        valid = (rr >= 0) & (rr < RES) & (cc >= 0) & (cc < RES)
        mask[idx[valid], (rr * RES + cc)[valid]] = 1.0
    mask[idx, idx] = 1.0
    return mask


def build_bass():
    import concourse.bacc as bacc
    import concourse.mybir as mybir
    from concourse.tile import TileContext
    from concourse.masks import make_identity
    from contextlib import ExitStack

    f32 = mybir.dt.float32
    u32 = mybir.dt.uint32
    AF = mybir.ActivationFunctionType

    nc = bacc.Bacc("TRN2", target_bir_lowering=False, debug=False, num_devices=NCORES)
    node = nc.declare_dram_parameter("node", [BPC, N, D], f32, isOutput=False)
    cbias = nc.declare_dram_parameter("cbias", [BPC, N, N], f32, isOutput=False)
    idx_out = nc.declare_dram_parameter("idx", [BPC, N, K], u32, isOutput=True)

    with TileContext(nc) as tc, ExitStack() as ctx:
        consts = ctx.enter_context(tc.tile_pool(name="consts", bufs=1))
        x_pool = ctx.enter_context(tc.tile_pool(name="x", bufs=2 * N_PT))
        xn_pool = ctx.enter_context(tc.tile_pool(name="xn", bufs=2 * N_PT))
        xnt_pool = ctx.enter_context(tc.tile_pool(name="xnt", bufs=8))
        ss_pool = ctx.enter_context(tc.tile_pool(name="ss", bufs=4))
        sq_scratch = ctx.enter_context(tc.tile_pool(name="sqscratch", bufs=3))
        cb_pool = ctx.enter_context(tc.tile_pool(name="cb", bufs=4))
        score_pool = ctx.enter_context(tc.tile_pool(name="score", bufs=4))
        small_pool = ctx.enter_context(tc.tile_pool(name="small", bufs=8))
        idx_pool = ctx.enter_context(tc.tile_pool(name="idx", bufs=4))
        ps_tr = ctx.enter_context(tc.tile_pool(name="ps_tr", bufs=4, space="PSUM"))
        ps_mm = ctx.enter_context(tc.tile_pool(name="ps_mm", bufs=4, space="PSUM"))

        ident = consts.tile([P, P], f32)
        make_identity(nc, ident)

        for b in range(BPC):
            # ---- load + row sum-of-squares ----
            x_t = []
            ss = ss_pool.tile([P, N_PT], f32, tag="ss")
            for j in range(N_PT):
                r = ROWS[j]
                xt = x_pool.tile([P, D], f32, tag="x")
                nc.sync.dma_start(out=xt[:r], in_=node.ap()[b, j * P : j * P + r, :])
                scr = sq_scratch.tile([P, D], f32, tag="sqscratch")
                nc.scalar.activation(
                    scr[:r], xt[:r], AF.Square, accum_out=ss[:r, j : j + 1]
                )
                x_t.append(xt)

            # ---- rinv = 1 / max(sqrt(ss), 1e-12) ----
            rinv = ss_pool.tile([P, N_PT], f32, tag="rinv")
            nc.scalar.activation(rinv, ss, AF.Sqrt)
            nc.vector.tensor_scalar_max(rinv, rinv, 1e-12)
            nc.vector.reciprocal(rinv, rinv)

            # ---- normalize: xh = x * rinv (per-partition scalar) ----
            xn_t = []
            for j in range(N_PT):
                r = ROWS[j]
                xnt = xn_pool.tile([P, D], f32, tag="xn")
                nc.scalar.activation(
                    xnt[:r], x_t[j][:r], AF.Copy, scale=rinv[:r, j : j + 1]
                )
                xn_t.append(xnt)

            # ---- transpose to [D, N] via PE transpose-mode ----
            xh_T = [
                xnt_pool.tile([P, N], f32, tag="xnt", name=f"xh_T_{b}_{k}")
                for k in range(4)
            ]
            for j in range(N_PT):
                r = ROWS[j]
                for k in range(4):
                    pst = ps_tr.tile([P, P], f32, tag="ps_tr")
                    nc.tensor.transpose(
                        pst[:, :r], xn_t[j][:r, k * P : (k + 1) * P], ident[:r, :r]
                    )
                    nc.scalar.activation(
                        xh_T[k][:, j * P : j * P + r], pst[:, :r], AF.Copy
                    )

            # ---- per row-tile: matmul, bias-subtract, top-10 ----
            for rt in range(N_PT):
                r = ROWS[rt]
                cb = cb_pool.tile([P, N], f32, tag="cb")
                nc.sync.dma_start(out=cb[:r], in_=cbias.ap()[b, rt * P : rt * P + r, :])
                score = score_pool.tile([P, N], f32, tag="score")
                for h in range(2):
                    ps = ps_mm.tile([P, HALF], f32, tag="ps_mm")
                    for k in range(4):
                        nc.tensor.matmul(
                            ps[:r],
                            lhsT=xh_T[k][:, rt * P : rt * P + r],
                            rhs=xh_T[k][:, h * HALF : (h + 1) * HALF],
                            start=(k == 0),
                            stop=(k == 3),
                        )
                    nc.vector.tensor_sub(
                        score[:r, h * HALF : (h + 1) * HALF],
                        ps[:r],
                        cb[:r, h * HALF : (h + 1) * HALF],
                    )

                idxt = idx_pool.tile([P, 16], u32, tag="idx")
                v1 = small_pool.tile([P, 8], f32, tag="v1")
                v2 = small_pool.tile([P, 8], f32, tag="v2")
                nc.vector.max(out=v1, in_=score)
                nc.vector.max_index(idxt[:, 0:8], v1, score)
                nc.vector.match_replace(
                    out=score, in_to_replace=v1, in_values=score, imm_value=-3.0e38
                )
                nc.vector.max(out=v2, in_=score)
                nc.vector.max_index(idxt[:, 8:16], v2, score)
                nc.sync.dma_start(
                    out=idx_out.ap()[b, rt * P : rt * P + r, :], in_=idxt[:r, 0:K]
                )

    nc.finalize()
    return nc


def _get_nc():
    if "nc" not in _CACHE:
        _CACHE["nc"] = build_bass()
    return _CACHE["nc"]


def kernel(node_feature, relative_pos):
    from concourse.bass_utils import run_bass_kernel_spmd

    x = np.asarray(node_feature, dtype=np.float32)
    rel = np.asarray(relative_pos, dtype=np.float32).reshape(N, N)

    # host prep: normalized-row squared norms + combined halved bias
    nrm = np.sqrt((x * x).sum(-1, dtype=np.float32), dtype=np.float32)
    nrm = np.maximum(nrm, np.float32(1e-12))
    xh = x / nrm[..., None]
    sq = (xh * xh).sum(-1, dtype=np.float32)  # [B, N]
    base = (rel + np.float32(INF) * _mask_np()).astype(np.float32)  # [N, N]
    cb = ((base[None] + sq[:, None, :]) * np.float32(0.5)).astype(np.float32)

    nc = _get_nc()
    in_maps = [
        {
            "node": np.ascontiguousarray(x[i * BPC : (i + 1) * BPC]),
            "cbias": np.ascontiguousarray(cb[i * BPC : (i + 1) * BPC]),
        }
        for i in range(NCORES)
    ]
    res = run_bass_kernel_spmd(nc, in_maps, list(range(NCORES)))
    topk = np.concatenate(
        [res.results[i]["idx"] for i in range(NCORES)], axis=0
    ).astype(np.int32)  # [B, N, K]

    dst = topk + (np.arange(BATCH, dtype=np.int32) * N)[:, None, None]
    src = np.broadcast_to(
        np.arange(BATCH * N, dtype=np.int32).reshape(BATCH, N, 1), (BATCH, N, K)
    )
    relation = np.zeros_like(dst)
    return np.stack([dst, src, relation], axis=-1).reshape(-1, 3)


# revision 6
# speedup vs baseline: 1.3351x; 1.3351x over previous
"""Trainium2 Bass kernel for nn_MediumRangeEdge (retrieval_knn).

For each batch graph: L2-normalize node features, pairwise distance
dist = sq_n + sq_m - 2*x@x.T + relative_pos + INF*mask, top-10 smallest
per node, emit edge list [dst, src, 0].

Distribution: data-parallel over batch. 32 graphs -> 8 NeuronCores, 4
graphs per core. No cross-device communication.

Device-side math per graph (n = query row, m = candidate column):
    score[n, m] = xh@xh.T[n, m] - cbias[n, m]
with host-precomputed cbias[b,n,m] = (rel[n,m] + INF*mask[n,m] + sq[b,m])/2
and host-precomputed rinv[b,n] = 1/max(||x_n||, 1e-12) (tiny aux inputs).
score = (-dist + sq_n)/2; the row-constant sq_n/2 leaves per-row order
unchanged, so top-10 of score == top-10 of -dist == jax.lax.top_k(-dist).
Top-10 per row on the DVE via max8 / max_index / match_replace (8+2).

Engine layout per core (4 graphs):
  ACT   normalize (x * rinv, rounded to matmul dtype)
  PE    28 transposes -> xh^T in [D, N]; 8 matmuls per 128-row tile
  DMA   loads + PSUM->SBUF copies
  POOL  score = mm - cbias (SBUF only)
  DVE   top-10: max8, max_index, match_replace, max8, max_index
"""

import sys

if "/opt/trn_rl_repo" not in sys.path:
    sys.path.insert(0, "/opt/trn_rl_repo")

import numpy as np

BATCH = 32
N = 784  # 28*28 nodes
D = 512
K = 10
RES = 28
INF = 100000.0
NCORES = 8
BPC = BATCH // NCORES  # graphs per core

P = 128
N_PT = 7  # partition tiles over N: 6*128 + 16
ROWS = [128, 128, 128, 128, 128, 128, 16]
HALF = 392  # free-dim split of N for PSUM (2 * 392 = 784)

# knobs
MM_DTYPE = "f32r"  # "f32" (exact, 4 cyc/row) or "f32r" (TF32-ish, 1 cyc/row)
SUB_ENGINE = "gpsimd"  # "dve" or "gpsimd" (via ACT PSUM->SBUF copy)

_CACHE = {}


def _mask_np():
    idx = np.arange(N)
    r, c = idx // RES, idx % RES
    mask = np.zeros((N, N), np.float32)
    for dr, dc in [(0, -1), (0, 1), (-1, 0), (1, 0), (-1, -1), (-1, 1), (1, -1), (1, 1)]:
        rr, cc = r + dr, c + dc
        valid = (rr >= 0) & (rr < RES) & (cc >= 0) & (cc < RES)
        mask[idx[valid], (rr * RES + cc)[valid]] = 1.0
    mask[idx, idx] = 1.0
    return mask


def build_bass():
    import concourse.bacc as bacc
    import concourse.mybir as mybir
    from concourse.tile import TileContext
    from concourse.masks import make_identity
    from contextlib import ExitStack

    f32 = mybir.dt.float32
    u32 = mybir.dt.uint32
    AF = mybir.ActivationFunctionType
    mmdt = mybir.dt.float32r if MM_DTYPE == "f32r" else f32

    nc = bacc.Bacc("TRN2", target_bir_lowering=False, debug=False, num_devices=NCORES)
    node = nc.declare_dram_parameter("node", [BPC, N, D], f32, isOutput=False)
    cbias = nc.declare_dram_parameter("cbias", [BPC, N, N], f32, isOutput=False)
    rinv_in = nc.declare_dram_parameter("rinv", [BPC, P, N_PT], f32, isOutput=False)
    idx_out = nc.declare_dram_parameter("idx", [BPC, N, K], u32, isOutput=True)

    with TileContext(nc) as tc, ExitStack() as ctx:
        consts = ctx.enter_context(tc.tile_pool(name="consts", bufs=1))
        x_pool = ctx.enter_context(tc.tile_pool(name="x", bufs=2 * N_PT))
        xn_pool = ctx.enter_context(tc.tile_pool(name="xn", bufs=2 * N_PT))
        xnt_pool = ctx.enter_context(tc.tile_pool(name="xnt", bufs=8))
        rv_pool = ctx.enter_context(tc.tile_pool(name="rv", bufs=4))
        cb_pool = ctx.enter_context(tc.tile_pool(name="cb", bufs=4))
        mm_pool = ctx.enter_context(tc.tile_pool(name="mm", bufs=6))
        score_pool = ctx.enter_context(tc.tile_pool(name="score", bufs=4))
        small_pool = ctx.enter_context(tc.tile_pool(name="small", bufs=8))
        idx_pool = ctx.enter_context(tc.tile_pool(name="idx", bufs=4))
        ps_tr = ctx.enter_context(tc.tile_pool(name="ps_tr", bufs=4, space="PSUM"))
        ps_mm = ctx.enter_context(tc.tile_pool(name="ps_mm", bufs=4, space="PSUM"))

        ident = consts.tile([P, P], f32)
        make_identity(nc, ident)
        if mmdt != f32:
            identr = consts.tile([P, P], mmdt)
            nc.scalar.activation(identr, ident, AF.Copy)
        else:
            identr = ident

        for b in range(BPC):
            rv = rv_pool.tile([P, N_PT], f32, tag="rv")
            nc.sync.dma_start(out=rv, in_=rinv_in.ap()[b])

            # ---- load + normalize (+ round to matmul dtype) ----
            xn_t = []
            for j in range(N_PT):
                r = ROWS[j]
                xt = x_pool.tile([P, D], f32, tag="x")
                nc.sync.dma_start(out=xt[:r], in_=node.ap()[b, j * P : j * P + r, :])
                xnt = xn_pool.tile([P, D], mmdt, tag="xn")
                nc.scalar.activation(
                    xnt[:r], xt[:r], AF.Copy, scale=rv[:r, j : j + 1]
                )
                xn_t.append(xnt)

            # ---- transpose to [D, N] via PE transpose-mode ----
            xh_T = [
                xnt_pool.tile([P, N], mmdt, tag="xnt", name=f"xh_T_{b}_{k}")
                for k in range(4)
            ]
            for j in range(N_PT):
                r = ROWS[j]
                for k in range(4):
                    pst = ps_tr.tile([P, P], mmdt, tag="ps_tr")
                    nc.tensor.transpose(
                        pst[:, :r], xn_t[j][:r, k * P : (k + 1) * P], identr[:r, :r]
                    )
                    nc.scalar.activation(
                        xh_T[k][:, j * P : j * P + r], pst[:, :r], AF.Copy
                    )

            # ---- per row-tile: matmul, bias-subtract, top-10 ----
            for rt in range(N_PT):
                r = ROWS[rt]
                cb = cb_pool.tile([P, N], f32, tag="cb")
                nc.sync.dma_start(out=cb[:r], in_=cbias.ap()[b, rt * P : rt * P + r, :])
                score = score_pool.tile([P, N], f32, tag="score")
                for h in range(2):
                    ps = ps_mm.tile([P, HALF], f32, tag="ps_mm")
                    for k in range(4):
                        nc.tensor.matmul(
                            ps[:r],
                            lhsT=xh_T[k][:, rt * P : rt * P + r],
                            rhs=xh_T[k][:, h * HALF : (h + 1) * HALF],
                            start=(k == 0),
                            stop=(k == 3),
                        )
                    sl = slice(h * HALF, (h + 1) * HALF)
                    if SUB_ENGINE == "gpsimd":
                        mm_sb = mm_pool.tile([P, HALF], f32, tag="mm")
                        nc.scalar.activation(mm_sb[:r], ps[:r], AF.Copy)
                        nc.gpsimd.tensor_sub(score[:r, sl], mm_sb[:r], cb[:r, sl])
                    else:
                        nc.vector.tensor_sub(score[:r, sl], ps[:r], cb[:r, sl])

                idxt = idx_pool.tile([P, 16], u32, tag="idx")
                v1 = small_pool.tile([P, 8], f32, tag="v1")
                v2 = small_pool.tile([P, 8], f32, tag="v2")
                nc.vector.max(out=v1, in_=score)
                nc.vector.max_index(idxt[:, 0:8], v1, score)
                nc.vector.match_replace(
                    out=score, in_to_replace=v1, in_values=score, imm_value=-3.0e38
                )
                nc.vector.max(out=v2, in_=score)
                nc.vector.max_index(idxt[:, 8:16], v2, score)
                nc.sync.dma_start(
                    out=idx_out.ap()[b, rt * P : rt * P + r, :], in_=idxt[:r, 0:K]
                )

    nc.finalize()
    return nc


def _get_nc():
    if "nc" not in _CACHE:
        _CACHE["nc"] = build_bass()
    return _CACHE["nc"]


def kernel(node_feature, relative_pos):
    from concourse.bass_utils import run_bass_kernel_spmd

    x = np.asarray(node_feature, dtype=np.float32)
    rel = np.asarray(relative_pos, dtype=np.float32).reshape(N, N)

    # host prep: normalization scales + combined halved bias (small aux data)
    nrm = np.sqrt((x * x).sum(-1, dtype=np.float32), dtype=np.float32)
    nrm = np.maximum(nrm, np.float32(1e-12))
    rinv = (np.float32(1.0) / nrm).astype(np.float32)  # [B, N]
    xh = x / nrm[..., None]
    sq = (xh * xh).sum(-1, dtype=np.float32)  # [B, N]
    base = (rel + np.float32(INF) * _mask_np()).astype(np.float32)  # [N, N]
    cb = ((base[None] + sq[:, None, :]) * np.float32(0.5)).astype(np.float32)

    # rinv laid out [B, 128, 7]: tile j, partition p -> node j*128+p (padded)
    rinv_pad = np.ones((BATCH, N_PT * P), np.float32)
    rinv_pad[:, :N] = rinv
    rinv_t = np.ascontiguousarray(
        rinv_pad.reshape(BATCH, N_PT, P).transpose(0, 2, 1)
    )

    nc = _get_nc()
    in_maps = [
        {
            "node": np.ascontiguousarray(x[i * BPC : (i + 1) * BPC]),
            "cbias": np.ascontiguousarray(cb[i * BPC : (i + 1) * BPC]),
            "rinv": np.ascontiguousarray(rinv_t[i * BPC : (i + 1) * BPC]),
        }
        for i in range(NCORES)
    ]
    res = run_bass_kernel_spmd(nc, in_maps, list(range(NCORES)))
    topk = np.concatenate(
        [res.results[i]["idx"] for i in range(NCORES)], axis=0
    ).astype(np.int32)  # [B, N, K]

    dst = topk + (np.arange(BATCH, dtype=np.int32) * N)[:, None, None]
    src = np.broadcast_to(
        np.arange(BATCH * N, dtype=np.int32).reshape(BATCH, N, 1), (BATCH, N, K)
    )
    relation = np.zeros_like(dst)
    return np.stack([dst, src, relation], axis=-1).reshape(-1, 3)


# revision 10
# speedup vs baseline: 1.3352x; 1.0000x over previous
"""Trainium2 Bass kernel for nn_MediumRangeEdge (retrieval_knn).

For each batch graph: L2-normalize node features, pairwise distance
dist = sq_n + sq_m - 2*x@x.T + relative_pos + INF*mask, top-10 smallest
per node, emit edge list [dst, src, 0].

Distribution: data-parallel over batch. 32 graphs -> 8 NeuronCores, 4
graphs per core. No cross-device communication.

Device-side math per graph (n = query row, m = candidate column):
    score[n, m] = xh@xh.T[n, m] - cbias[n, m]
with host-precomputed cbias[b,n,m] = (rel[n,m] + INF*mask[n,m] + sq[b,m])/2
and host-precomputed rinv[b,n] = 1/max(||x_n||, 1e-12) (tiny aux inputs).
score = (-dist + sq_n)/2; the row-constant sq_n/2 leaves per-row order
unchanged, so top-10 of score == top-10 of -dist == jax.lax.top_k(-dist).
Top-10 per row on the DVE via max8 / max_index / match_replace (8+2).

Engine layout per core (4 graphs):
  ACT   normalize (x * rinv, rounded to matmul dtype)
  PE    28 transposes -> xh^T in [D, N]; 8 matmuls per 128-row tile
  DMA   loads + PSUM->SBUF copies
  POOL  score = mm - cbias (SBUF only)
  DVE   top-10: max8, max_index, match_replace, max8, max_index
"""

import sys

if "/opt/trn_rl_repo" not in sys.path:
    sys.path.insert(0, "/opt/trn_rl_repo")

import numpy as np

BATCH = 32
N = 784  # 28*28 nodes
D = 512
K = 10
RES = 28
INF = 100000.0
NCORES = 8
BPC = BATCH // NCORES  # graphs per core

P = 128
N_PT = 7  # partition tiles over N: 6*128 + 16
ROWS = [128, 128, 128, 128, 128, 128, 16]
HALVES = [(0, 384), (384, 400)]  # column split of N; 128-aligned so lhsT never crosses

# knobs
MM_DTYPE = "f32r"  # "f32" (exact, 4 cyc/row) or "f32r" (TF32-ish, 1 cyc/row)
SUB_ENGINE = "gpsimd"  # "dve" or "gpsimd" (via ACT PSUM->SBUF copy)
BUFS = dict(x=14, xn=14, xnt=16, rv=4, cb=6, mm=6, score=6, small=12, idx=6,
            ps_tr=4, ps_mm=4)

_CACHE = {}


def _mask_np():
    idx = np.arange(N)
    r, c = idx // RES, idx % RES
    mask = np.zeros((N, N), np.float32)
    for dr, dc in [(0, -1), (0, 1), (-1, 0), (1, 0), (-1, -1), (-1, 1), (1, -1), (1, 1)]:
        rr, cc = r + dr, c + dc
        valid = (rr >= 0) & (rr < RES) & (cc >= 0) & (cc < RES)
        mask[idx[valid], (rr * RES + cc)[valid]] = 1.0
    mask[idx, idx] = 1.0
    return mask


def build_bass():
    import concourse.bacc as bacc
    import concourse.mybir as mybir
    from concourse.tile import TileContext
    from concourse.masks import make_identity
    from contextlib import ExitStack

    f32 = mybir.dt.float32
    u32 = mybir.dt.uint32
    AF = mybir.ActivationFunctionType
    mmdt = mybir.dt.float32r if MM_DTYPE == "f32r" else f32

    nc = bacc.Bacc("TRN2", target_bir_lowering=False, debug=False, num_devices=NCORES)
    node = nc.declare_dram_parameter("node", [BPC, N, D], f32, isOutput=False)
    cbias = nc.declare_dram_parameter("cbias", [BPC, N, N], f32, isOutput=False)
    rinv_in = nc.declare_dram_parameter("rinv", [BPC, P, N_PT], f32, isOutput=False)
    idx_out = nc.declare_dram_parameter("idx", [BPC, N, K], u32, isOutput=True)

    with TileContext(nc) as tc, ExitStack() as ctx:
        consts = ctx.enter_context(tc.tile_pool(name="consts", bufs=1))
        x_pool = ctx.enter_context(tc.tile_pool(name="x", bufs=BUFS["x"]))
        xn_pool = ctx.enter_context(tc.tile_pool(name="xn", bufs=BUFS["xn"]))
        xnt_pool = ctx.enter_context(tc.tile_pool(name="xnt", bufs=BUFS["xnt"]))
        rv_pool = ctx.enter_context(tc.tile_pool(name="rv", bufs=BUFS["rv"]))
        cb_pool = ctx.enter_context(tc.tile_pool(name="cb", bufs=BUFS["cb"]))
        mm_pool = ctx.enter_context(tc.tile_pool(name="mm", bufs=BUFS["mm"]))
        score_pool = ctx.enter_context(tc.tile_pool(name="score", bufs=BUFS["score"]))
        small_pool = ctx.enter_context(tc.tile_pool(name="small", bufs=BUFS["small"]))
        idx_pool = ctx.enter_context(tc.tile_pool(name="idx", bufs=BUFS["idx"]))
        ps_tr = ctx.enter_context(tc.tile_pool(name="ps_tr", bufs=BUFS["ps_tr"], space="PSUM"))
        ps_mm = ctx.enter_context(tc.tile_pool(name="ps_mm", bufs=BUFS["ps_mm"], space="PSUM"))

        ident = consts.tile([P, P], f32)
        make_identity(nc, ident)
        if mmdt != f32:
            identr = consts.tile([P, P], mmdt)
            nc.scalar.activation(identr, ident, AF.Copy)
        else:
            identr = ident

        for b in range(BPC):
            rv = rv_pool.tile([P, N_PT], f32, tag="rv")
            nc.sync.dma_start(out=rv, in_=rinv_in.ap()[b])

            # ---- load + normalize (+ round to matmul dtype) ----
            xn_t = []
            for j in range(N_PT):
                r = ROWS[j]
                xt = x_pool.tile([P, D], f32, tag="x")
                nc.sync.dma_start(out=xt[:r], in_=node.ap()[b, j * P : j * P + r, :])
                xnt = xn_pool.tile([P, D], mmdt, tag="xn")
                nc.scalar.activation(
                    xnt[:r], xt[:r], AF.Copy, scale=rv[:r, j : j + 1]
                )
                xn_t.append(xnt)

            # ---- transpose to [D, N] via PE transpose-mode ----
            # xh_T split into 2 column-halves so first matmuls start after
            # only the first 3 node-tiles are transposed.
            xh_T = [
                [
                    xnt_pool.tile(
                        [P, hw], mmdt, tag=f"xnt{hi}", name=f"xh_T_{b}_{k}_{hi}"
                    )
                    for hi, (h0, hw) in enumerate(HALVES)
                ]
                for k in range(4)
            ]
            for j in range(N_PT):
                r = ROWS[j]
                hi = 0 if (j + 1) * P <= 384 else 1
                h0 = HALVES[hi][0]
                for k in range(4):
                    pst = ps_tr.tile([P, P], mmdt, tag="ps_tr")
                    nc.tensor.transpose(
                        pst[:, :r], xn_t[j][:r, k * P : (k + 1) * P], identr[:r, :r]
                    )
                    nc.scalar.activation(
                        xh_T[k][hi][:, j * P - h0 : j * P - h0 + r], pst[:, :r], AF.Copy
                    )

            # ---- per row-tile: matmul, bias-subtract, top-10 ----
            for rt in range(N_PT):
                r = ROWS[rt]
                cb = cb_pool.tile([P, N], f32, tag="cb")
                nc.sync.dma_start(out=cb[:r], in_=cbias.ap()[b, rt * P : rt * P + r, :])
                score = score_pool.tile([P, N], f32, tag="score")
                lhs_hi = 0 if (rt + 1) * P <= 384 else 1
                lhs_off = rt * P - HALVES[lhs_hi][0]
                for h, (h0, hw) in enumerate(HALVES):
                    ps = ps_mm.tile([P, 400], f32, tag="ps_mm")
                    for k in range(4):
                        nc.tensor.matmul(
                            ps[:r, :hw],
                            lhsT=xh_T[k][lhs_hi][:, lhs_off : lhs_off + r],
                            rhs=xh_T[k][h],
                            start=(k == 0),
                            stop=(k == 3),
                        )
                    sl = slice(h0, h0 + hw)
                    if SUB_ENGINE == "gpsimd":
                        mm_sb = mm_pool.tile([P, 400], f32, tag="mm")
                        nc.scalar.activation(mm_sb[:r, :hw], ps[:r, :hw], AF.Copy)
                        nc.gpsimd.tensor_sub(
                            score[:r, sl], mm_sb[:r, :hw], cb[:r, sl]
                        )
                    else:
                        nc.vector.tensor_sub(score[:r, sl], ps[:r, :hw], cb[:r, sl])

                idxt = idx_pool.tile([P, 16], u32, tag="idx")
                v1 = small_pool.tile([P, 8], f32, tag="v1")
                v2 = small_pool.tile([P, 8], f32, tag="v2")
                nc.vector.max(out=v1, in_=score)
                nc.vector.max_index(idxt[:, 0:8], v1, score)
                nc.vector.match_replace(
                    out=score, in_to_replace=v1, in_values=score, imm_value=-3.0e38
                )
                nc.vector.max(out=v2, in_=score)
                nc.vector.max_index(idxt[:, 8:16], v2, score)
                nc.sync.dma_start(
                    out=idx_out.ap()[b, rt * P : rt * P + r, :], in_=idxt[:r, 0:K]
                )

    nc.finalize()
    return nc


def _get_nc():
    if "nc" not in _CACHE:
        _CACHE["nc"] = build_bass()
    return _CACHE["nc"]


def kernel(node_feature, relative_pos):
    from concourse.bass_utils import run_bass_kernel_spmd

    x = np.asarray(node_feature, dtype=np.float32)
    rel = np.asarray(relative_pos, dtype=np.float32).reshape(N, N)

    # host prep: normalization scales + combined halved bias (small aux data)
    nrm = np.sqrt((x * x).sum(-1, dtype=np.float32), dtype=np.float32)
    nrm = np.maximum(nrm, np.float32(1e-12))
    rinv = (np.float32(1.0) / nrm).astype(np.float32)  # [B, N]
    xh = x / nrm[..., None]
    sq = (xh * xh).sum(-1, dtype=np.float32)  # [B, N]
    base = (rel + np.float32(INF) * _mask_np()).astype(np.float32)  # [N, N]
    cb = ((base[None] + sq[:, None, :]) * np.float32(0.5)).astype(np.float32)

    # rinv laid out [B, 128, 7]: tile j, partition p -> node j*128+p (padded)
    rinv_pad = np.ones((BATCH, N_PT * P), np.float32)
    rinv_pad[:, :N] = rinv
    rinv_t = np.ascontiguousarray(
        rinv_pad.reshape(BATCH, N_PT, P).transpose(0, 2, 1)
    )

    nc = _get_nc()
    in_maps = [
        {
            "node": np.ascontiguousarray(x[i * BPC : (i + 1) * BPC]),
            "cbias": np.ascontiguousarray(cb[i * BPC : (i + 1) * BPC]),
            "rinv": np.ascontiguousarray(rinv_t[i * BPC : (i + 1) * BPC]),
        }
        for i in range(NCORES)
    ]
    res = run_bass_kernel_spmd(nc, in_maps, list(range(NCORES)))
    topk = np.concatenate(
        [res.results[i]["idx"] for i in range(NCORES)], axis=0
    ).astype(np.int32)  # [B, N, K]

    dst = topk + (np.arange(BATCH, dtype=np.int32) * N)[:, None, None]
    src = np.broadcast_to(
        np.arange(BATCH * N, dtype=np.int32).reshape(BATCH, N, 1), (BATCH, N, K)
    )
    relation = np.zeros_like(dst)
    return np.stack([dst, src, relation], axis=-1).reshape(-1, 3)
